# revision 11
# baseline (speedup 1.0000x reference)
"""Bass/Trainium2 kernel for masked attention + resize (nn_BaseAttender).

Full-input contract: kernel(**inputs) takes the complete unsharded tensors,
shards batch-wise across 8 NeuronCores (2 batches per core), runs one SPMD
Bass program, and gathers the full [16, 1024, 256] output.

Math (per batch):
    logits  = Q @ K^T / sqrt(512)              [1024, 2048]
    attn    = softmax(where(mask==0, -1e9, logits))
    context = attn @ V                          [1024, 512]
    out     = context @ W^T + b                 [1024, 256]

Implementation notes:
  - all heavy-lift layout work is done on the HOST: Q^T, K^T, mask^T are
    pre-transposed and pre-cast to bf16 so the device does ZERO PE
    transposes and loads exactly the tiles it consumes via big contiguous
    DMAs.  This kernel's PE time is ~pure GEMM.
  - scores are computed directly in [k, q] layout (stationary = K^T tile,
    moving = Q^T), so exp*mask is already in the layout phase 2 needs.
  - softmax without max-subtraction: logits are O(5) so exp() is safe, and
    where(mask==0,-inf)+softmax == exp(logits)*mask normalized by its sum.
  - denominator: a pairwise bf16 add-tree on the DVE accumulates
    sum_kt exp tiles (GPSIMD shares SBUF ports with the DVE and slows it
    4x, so the tree must stay on the DVE); a 1-moving-column fp32 matmul
    per q-tile (stationary = acc slice, moving = ones) reduces over
    partitions directly into [q, 1] layout.
  - phase 3 (resize) is interleaved into phase 2 per 512-query chunk so
    its small PSUM groups hide behind phase-2 streaming.
  - a warmup matmul group runs during the input-DMA wait to bring the PE
    out of its low p-state before the first real matmul.
  - the 1/denominator scaling commutes past the k- and v-contractions and
    is applied once at the very end on [q, 256] tiles.
"""

import sys

sys.path.insert(0, "/opt/trn_rl_repo")

import numpy as np

import concourse.bass as bass
import concourse.tile as tile
from concourse import bacc, mybir
from concourse.bass_utils import run_bass_kernel_spmd

# problem shape (hardcoded per contract)
B, NQ, NK, D, V, O = 16, 1024, 2048, 512, 512, 256
N_CORES = 8
B_LOC = B // N_CORES          # batches per core
SCALE = 1.0 / np.sqrt(np.float32(512.0))

P = 128
DT = D // P                   # 4 d-tiles (contraction of phase 1)
KT = NK // P                  # 16 k-tiles
QT = NQ // P                  # 8 q-tiles
QC = NQ // 512                # 2 q-chunks of 512 (phase-1 moving dim)
VT = V // P                   # 4 v-tiles
MC = 2                        # k-tiles per mask DMA chunk
NMC = KT // MC                # 8 mask chunks per batch
KH = KT // 2                  # k-tiles per K^T half-tile

F32 = mybir.dt.float32
BF = mybir.dt.bfloat16

_NC_CACHE = {}
_LAST_RESULTS = {}

ADD = mybir.AluOpType.add
MULT = mybir.AluOpType.mult


def _build():
    nc = bacc.Bacc(num_swdge_queues=2)
    k_t = nc.declare_dram_parameter("k_t", [B_LOC, P, KT, DT, P], BF, isOutput=False)
    q_t = nc.declare_dram_parameter("q_t", [B_LOC, P, QC, DT, 512], BF, isOutput=False)
    v_t = nc.declare_dram_parameter("v_t", [B_LOC, P, KT, V], BF, isOutput=False)
    m_t = nc.declare_dram_parameter("m_t", [B_LOC, P, KT, NQ], BF, isOutput=False)
    w_t = nc.declare_dram_parameter("w_t", [P, VT, O], BF, isOutput=False)
    b_r = nc.declare_dram_parameter("b_resize", [P, O], F32, isOutput=False)
    out = nc.declare_dram_parameter("out", [B_LOC, NQ, O], F32, isOutput=True)

    with tile.TileContext(nc) as tc:
        with (
            tc.tile_pool(name="const", bufs=1) as constp,
            tc.tile_pool(name="kt_sb", bufs=3) as ktp,
            tc.tile_pool(name="qt_sb", bufs=2) as qtp,
            tc.tile_pool(name="v_sb", bufs=2) as vp,
            tc.tile_pool(name="m_sb", bufs=3) as mp,
            tc.tile_pool(name="etmp", bufs=6) as etp,      # exp staging + L1 partials
            tc.tile_pool(name="l2p", bufs=2) as l2p,
            tc.tile_pool(name="l3p", bufs=2) as l3p,
            tc.tile_pool(name="expt", bufs=2) as exp_p,
            tc.tile_pool(name="acc", bufs=2) as accp,
            tc.tile_pool(name="ctxt", bufs=1) as ctp,
            tc.tile_pool(name="recips", bufs=2) as rcp,
            tc.tile_pool(name="outsb", bufs=4) as osp,
            tc.tile_pool(name="ps_s", bufs=2, space="PSUM") as psp,    # phase-1 scores
            tc.tile_pool(name="ps_c", bufs=2, space="PSUM") as pcp,    # phase-2 context
            tc.tile_pool(name="ps_o", bufs=2, space="PSUM") as pop,    # out + den + warmup
        ):
            w_sb = constp.tile([P, VT, O], BF)
            ones_bf = constp.tile([P, 1], BF)
            nc.vector.memset(ones_bf[:], 1.0)
            ones_sb = constp.tile([P, 1], F32)
            nc.vector.memset(ones_sb[:], 1.0)
            warmdata = constp.tile([P, 512], BF)
            nc.vector.memset(warmdata[:], 0.0)

            # PE warmup during the input-DMA wait: one long accumulation
            # group streaming a memset tile through the array (result
            # discarded, no DMA dependency).
            warm = pop.tile([P, 2, O], F32, tag="out")
            NWARM = 16
            for i in range(NWARM):
                nc.tensor.matmul(
                    warm[0:1, :, :], ones_bf[:], warmdata[:],
                    start=(i == 0), stop=(i == NWARM - 1),
                )

            bias_sb = constp.tile([P, O], F32)

            ks, qts, vs, ms = {}, {}, {}, {}

            def _m_chunk(b, kt0, n, eng):
                mrow = mp.tile([P, MC, NQ], BF, tag="m", name=f"m_{b}_{kt0}")
                eng.dma_start(mrow[:, 0:n], m_t[b, :, kt0:kt0 + n])
                for i in range(n):
                    ms[b].append(mrow[:, i, :])

            def stage0():
                # batch 0: spread the critical first loads across idle engine
                # queues so their issue overheads overlap, and interleave the
                # remaining mask chunks between the K/V loads so every tile
                # lands just before its first consumer.
                b = 0
                ms[b] = []
                kh0 = ktp.tile([P, KH, DT, P], BF, tag="k", name=f"k_{b}_0")
                nc.sync.dma_start(kh0[:, 0:4], k_t[b, :, 0:4])
                qt_sb = qtp.tile([P, QC, DT, 512], BF, tag="q", name=f"q_{b}")
                nc.scalar.dma_start(qt_sb[:, 0], q_t[b, :, 0])
                nc.scalar.dma_start(qt_sb[:, 1], q_t[b, :, 1])
                _m_chunk(b, 0, 1, nc.gpsimd)
                _m_chunk(b, 1, 1, nc.gpsimd)
                nc.sync.dma_start(kh0[:, 4:KH], k_t[b, :, 4:KH])
                _m_chunk(b, 2, 2, nc.sync)
                kh1 = ktp.tile([P, KH, DT, P], BF, tag="k", name=f"k_{b}_1")
                nc.sync.dma_start(kh1[:], k_t[b, :, KH:KT])
                # everything below sits behind pool-gated mask chunks in the
                # sync FIFO, so these transfers cannot start before ph1(0) is
                # underway — the DMA rings packet-interleave all active
                # streams, so issuing bulk loads early would starve the
                # critical first tiles.
                _m_chunk(b, 4, 2, nc.sync)
                v_sb = vp.tile([P, KT, V], BF, tag="v", name=f"v_{b}")
                nc.sync.dma_start(v_sb[:, 0:KH], v_t[b, :, 0:KH])
                _m_chunk(b, 6, 2, nc.sync)
                nc.sync.dma_start(v_sb[:, KH:KT], v_t[b, :, KH:KT])
                _m_chunk(b, 8, 2, nc.sync)
                nc.sync.dma_start(w_sb[:], w_t[:])
                nc.sync.dma_start(bias_sb[:], b_r[:])
                for kt0 in range(10, KT, 2):
                    _m_chunk(b, kt0, 2, nc.sync)
                ks[b], qts[b], vs[b] = (kh0, kh1), qt_sb, v_sb

            def stage1():
                # batch 1: bulk loads first (they have until ph1(1)/ph2(1)),
                # then mask chunks, which self-pace through the pool slots.
                b = 1
                ms[b] = []
                kh0 = ktp.tile([P, KH, DT, P], BF, tag="k", name=f"k_{b}_0")
                nc.sync.dma_start(kh0[:], k_t[b, :, 0:KH])
                qt_sb = qtp.tile([P, QC, DT, 512], BF, tag="q", name=f"q_{b}")
                nc.sync.dma_start(qt_sb[:], q_t[b])
                kh1 = ktp.tile([P, KH, DT, P], BF, tag="k", name=f"k_{b}_1")
                nc.sync.dma_start(kh1[:], k_t[b, :, KH:KT])
                v_sb = vp.tile([P, KT, V], BF, tag="v", name=f"v_{b}")
                nc.sync.dma_start(v_sb[:], v_t[b])
                for kt0 in range(0, KT, 2):
                    _m_chunk(b, kt0, 2, nc.sync)
                ks[b], qts[b], vs[b] = (kh0, kh1), qt_sb, v_sb

            state = {}

            def ph1(b):
                khs, qt_sb, mrows = ks[b], qts[b], ms[b]
                expt = exp_p.tile([P, KT, NQ], BF, tag="expt", name=f"expt_{b}")
                acc = accp.tile([P, NQ], F32, tag="acc", name=f"acc_{b}")
                levels = {}

                def tree_push(lv, t):
                    # pairwise DVE add-tree over exp tiles, bf16 until the
                    # final fp32 combine into acc
                    while lv in levels:
                        t2 = levels.pop(lv)
                        if lv == 3:
                            nc.vector.tensor_tensor(acc[:], t2, t, ADD)
                            return
                        pool = etp if lv == 0 else (l2p if lv == 1 else l3p)
                        nt = pool.tile([P, NQ], BF, tag="etmp" if lv == 0 else "p")
                        nc.vector.tensor_tensor(nt[:], t2, t, ADD)
                        t, lv = nt[:], lv + 1
                    levels[lv] = t

                for kt in range(KT):
                    kh = khs[kt // KH]
                    ps = psp.tile([P, QC, 512], F32, tag="scores")
                    for dt in range(DT):
                        for qc in range(QC):
                            nc.tensor.matmul(
                                ps[:, qc, :],
                                kh[:, kt % KH, dt, :],
                                qt_sb[:, qc, dt, :],
                                start=(dt == 0),
                                stop=(dt == DT - 1),
                            )
                    et = etp.tile([P, NQ], BF, tag="etmp")
                    for qc in range(QC):
                        nc.scalar.activation(
                            et[:, qc * 512:(qc + 1) * 512], ps[:, qc, :],
                            mybir.ActivationFunctionType.Exp, scale=float(SCALE),
                        )
                    nc.vector.tensor_tensor(expt[:, kt, :], et[:], mrows[kt], MULT)
                    tree_push(0, expt[:, kt, :])
                state[b] = (expt, acc)

            def den_recip(b):
                acc = state[b][1]
                den_ps = pop.tile([P, 2, O], F32, tag="out")
                for qt in range(QT):
                    nc.tensor.matmul(
                        den_ps[0:P, 0, qt:qt + 1],
                        acc[:, qt * P:(qt + 1) * P],
                        ones_sb[:],
                        start=True,
                        stop=True,
                    )
                recips = rcp.tile([P, QT], F32, tag="recips", name=f"recips_{b}")
                nc.vector.reciprocal(recips[:], den_ps[:, 0, 0:QT])
                state[b] = (state[b][0], recips)

            def ph2_qc(b, qc, ctxt):
                expt = state[b][0]
                v_sb = vs[b]
                for vt in range(VT):
                    ps_c = pcp.tile([P, 512], F32, tag="ctx")
                    for kt in range(KT):
                        nc.tensor.matmul(
                            ps_c[:],
                            v_sb[:, kt, vt * P:(vt + 1) * P],
                            expt[:, kt, qc * 512:(qc + 1) * 512],
                            start=(kt == 0),
                            stop=(kt == KT - 1),
                        )
                    nc.vector.tensor_copy(ctxt[:, vt, qc * 512:(qc + 1) * 512], ps_c[:])

            def ph3_half(b, qc, ctxt):
                recips = state[b][1]
                for qh in range(QT // QC // 2):  # 2 qt-pairs per q-chunk
                    ps_o = pop.tile([P, 2, O], F32, tag="out")
                    for s in range(2):
                        qt = qc * (QT // QC) + qh * 2 + s
                        for vt in range(VT):
                            nc.tensor.matmul(
                                ps_o[:, s, :],
                                ctxt[:, vt, qt * P:(qt + 1) * P],
                                w_sb[:, vt, :],
                                start=(vt == 0),
                                stop=(vt == VT - 1),
                            )
                        out_t = osp.tile([P, O], F32, tag="out_sb")
                        nc.vector.scalar_tensor_tensor(
                            out_t[:], ps_o[:, s, :], recips[:, qt:qt + 1], bias_sb[:],
                            MULT, ADD,
                        )
                        nc.gpsimd.dma_start(out[b, qt * P:(qt + 1) * P, :], out_t[:])

            def ph23(b):
                ctxt = ctp.tile([P, VT, NQ], BF, tag="ctxt", name=f"ctxt_{b}")
                ph2_qc(b, 0, ctxt)
                den_recip(b)
                ph3_half(b, 0, ctxt)
                ph2_qc(b, 1, ctxt)
                ph3_half(b, 1, ctxt)

            stage0()
            stage1()
            ph1(0)
            ph23(0)
            ph1(1)
            ph23(1)

    nc.finalize()
    return nc


def _prep(keys, queries, values, mask, W_resize, b_resize):
    bf = mybir.dt.np(BF)
    k_bf = np.asarray(keys, dtype=np.float32).astype(bf)
    q_bf = np.asarray(queries, dtype=np.float32).astype(bf)
    v_bf = np.asarray(values, dtype=np.float32).astype(bf)
    m_bf = np.asarray(mask).astype(bf)
    w_bf = np.asarray(W_resize, dtype=np.float32).astype(bf)

    # k_t[b, p, kt, dt, c] = K[b, kt*128+c, dt*128+p]
    k_t = np.ascontiguousarray(
        k_bf.reshape(B, KT, P, DT, P).transpose(0, 4, 1, 3, 2)
    )
    # q_t[b, p, qc, dt, j] = Q[b, qc*512+j, dt*128+p]
    q_t = np.ascontiguousarray(
        q_bf.reshape(B, QC, 512, DT, P).transpose(0, 4, 1, 3, 2)
    )
    # v_t[b, p, kt, v] = V[b, kt*128+p, v]
    v_t = np.ascontiguousarray(v_bf.reshape(B, KT, P, V).transpose(0, 2, 1, 3))
    # m_t[b, p, kt, q] = mask[b, q, kt*128+p]
    m_t = np.ascontiguousarray(m_bf.reshape(B, NQ, KT, P).transpose(0, 3, 2, 1))
    # w_t[p, vt, o] = W[o, vt*128+p]
    w_t = np.ascontiguousarray(w_bf.reshape(O, VT, P).transpose(2, 1, 0))
    b_rep = np.ascontiguousarray(
        np.broadcast_to(np.asarray(b_resize, dtype=np.float32).reshape(1, O), (P, O))
    )
    return k_t, q_t, v_t, m_t, w_t, b_rep


def kernel(keys, queries, values, mask, W_resize, b_resize):
    k_t, q_t, v_t, m_t, w_t, b_rep = _prep(
        keys, queries, values, mask, W_resize, b_resize
    )

    if "nc" not in _NC_CACHE:
        _NC_CACHE["nc"] = _build()
    nc = _NC_CACHE["nc"]

    in_maps = []
    for c in range(N_CORES):
        s = slice(c * B_LOC, (c + 1) * B_LOC)
        in_maps.append(
            {
                "k_t": k_t[s],
                "q_t": q_t[s],
                "v_t": v_t[s],
                "m_t": m_t[s],
                "w_t": w_t,
                "b_resize": b_rep,
            }
        )

    r = run_bass_kernel_spmd(nc, in_maps, list(range(N_CORES)))
    _LAST_RESULTS["r"] = r
    return np.concatenate([r.results[c]["out"] for c in range(N_CORES)], axis=0)


# revision 12
# speedup vs baseline: 1.0456x; 1.0456x over previous
"""Bass/Trainium2 kernel for masked attention + resize (nn_BaseAttender).

Full-input contract: kernel(**inputs) takes the complete unsharded tensors,
shards batch-wise across 8 NeuronCores (2 batches per core), runs one SPMD
Bass program, and gathers the full [16, 1024, 256] output.

Math (per batch):
    logits  = Q @ K^T / sqrt(512)              [1024, 2048]
    attn    = softmax(where(mask==0, -1e9, logits))
    context = attn @ V                          [1024, 512]
    out     = context @ W^T + b                 [1024, 256]

Implementation notes:
  - all heavy-lift layout work is done on the HOST: Q^T, K^T, mask^T are
    pre-transposed and pre-cast to bf16 so the device does ZERO PE
    transposes and loads exactly the tiles it consumes via big contiguous
    DMAs.  This kernel's PE time is ~pure GEMM.
  - scores are computed directly in [k, q] layout (stationary = K^T tile,
    moving = Q^T), so exp*mask is already in the layout phase 2 needs.
  - softmax without max-subtraction: logits are O(5) so exp() is safe, and
    where(mask==0,-inf)+softmax == exp(logits)*mask normalized by its sum.
  - denominator: a pairwise bf16 add-tree on the DVE accumulates
    sum_kt exp tiles (GPSIMD shares SBUF ports with the DVE and slows it
    4x, so the tree must stay on the DVE); a 1-moving-column fp32 matmul
    per q-tile (stationary = acc slice, moving = ones) reduces over
    partitions directly into [q, 1] layout.
  - phase 3 (resize) is interleaved into phase 2 per 512-query chunk so
    its small PSUM groups hide behind phase-2 streaming.
  - a warmup matmul group runs during the input-DMA wait to bring the PE
    out of its low p-state before the first real matmul.
  - the 1/denominator scaling commutes past the k- and v-contractions and
    is applied once at the very end on [q, 256] tiles.
"""

import sys

sys.path.insert(0, "/opt/trn_rl_repo")

import numpy as np

import concourse.bass as bass
import concourse.tile as tile
from concourse import bacc, mybir
from concourse.bass_utils import run_bass_kernel_spmd

# problem shape (hardcoded per contract)
B, NQ, NK, D, V, O = 16, 1024, 2048, 512, 512, 256
N_CORES = 8
B_LOC = B // N_CORES          # batches per core
SCALE = 1.0 / np.sqrt(np.float32(512.0))

P = 128
DT = D // P                   # 4 d-tiles (contraction of phase 1)
KT = NK // P                  # 16 k-tiles
QT = NQ // P                  # 8 q-tiles
QC = NQ // 512                # 2 q-chunks of 512 (phase-1 moving dim)
VT = V // P                   # 4 v-tiles
MC = 2                        # k-tiles per mask DMA chunk
NMC = KT // MC                # 8 mask chunks per batch
KH = KT // 2                  # k-tiles per K^T half-tile

F32 = mybir.dt.float32
BF = mybir.dt.bfloat16

_NC_CACHE = {}
_LAST_RESULTS = {}

ADD = mybir.AluOpType.add
MULT = mybir.AluOpType.mult


def _build():
    nc = bacc.Bacc(num_swdge_queues=2)
    k_t = nc.declare_dram_parameter("k_t", [B_LOC, P, KT, DT, P], BF, isOutput=False)
    q_t = nc.declare_dram_parameter("q_t", [B_LOC, P, QC, DT, 512], BF, isOutput=False)
    v_t = nc.declare_dram_parameter("v_t", [B_LOC, P, KT, V], BF, isOutput=False)
    m_t = nc.declare_dram_parameter("m_t", [B_LOC, P, KT, NQ], BF, isOutput=False)
    w_t = nc.declare_dram_parameter("w_t", [P, VT, O], BF, isOutput=False)
    b_r = nc.declare_dram_parameter("b_resize", [P, O], F32, isOutput=False)
    out = nc.declare_dram_parameter("out", [B_LOC, NQ, O], F32, isOutput=True)

    with tile.TileContext(nc) as tc:
        with (
            tc.tile_pool(name="const", bufs=1) as constp,
            tc.tile_pool(name="kt_sb", bufs=3) as ktp,
            tc.tile_pool(name="qt_sb", bufs=2) as qtp,
            tc.tile_pool(name="v_sb", bufs=2) as vp,
            tc.tile_pool(name="m_sb", bufs=3) as mp,
            tc.tile_pool(name="etmp", bufs=6) as etp,      # exp staging + L1 partials
            tc.tile_pool(name="l2p", bufs=2) as l2p,
            tc.tile_pool(name="l3p", bufs=2) as l3p,
            tc.tile_pool(name="expt", bufs=2) as exp_p,
            tc.tile_pool(name="acc", bufs=2) as accp,
            tc.tile_pool(name="ctxt", bufs=1) as ctp,
            tc.tile_pool(name="recips", bufs=2) as rcp,
            tc.tile_pool(name="outsb", bufs=4) as osp,
            tc.tile_pool(name="ps_s", bufs=2, space="PSUM") as psp,    # phase-1 scores
            tc.tile_pool(name="ps_c", bufs=2, space="PSUM") as pcp,    # phase-2 context
            tc.tile_pool(name="ps_o", bufs=2, space="PSUM") as pop,    # out + den + warmup
        ):
            w_sb = constp.tile([P, VT, O], BF)
            ones_bf = constp.tile([P, 1], BF)
            nc.vector.memset(ones_bf[:], 1.0)
            ones_sb = constp.tile([P, 1], F32)
            nc.vector.memset(ones_sb[:], 1.0)
            warmdata = constp.tile([P, 512], BF)
            nc.vector.memset(warmdata[:], 0.0)

            # PE warmup during the input-DMA wait: one long accumulation
            # group streaming a memset tile through the array (result
            # discarded, no DMA dependency).
            warm = pop.tile([P, 2, O], F32, tag="out")
            NWARM = 16
            for i in range(NWARM):
                nc.tensor.matmul(
                    warm[0:1, :, :], ones_bf[:], warmdata[:],
                    start=(i == 0), stop=(i == NWARM - 1),
                )

            bias_sb = constp.tile([P, O], F32)

            ks, qts, vs, ms = {}, {}, {}, {}

            def _m_chunk(b, kt0):
                mrow = mp.tile([P, MC, NQ], BF, tag="m", name=f"m_{b}_{kt0}")
                nc.sync.dma_start(mrow[:], m_t[b, :, kt0:kt0 + MC])
                for i in range(MC):
                    ms[b].append(mrow[:, i, :])

            # All input loads go through the single sync queue in strict
            # need order: same-queue transfers serialize in order, while
            # transfers from different queues packet-interleave on the DMA
            # rings (fair share) and would starve the critical first tiles.
            def stage0():
                b = 0
                ms[b] = []
                kh0 = ktp.tile([P, KH, DT, P], BF, tag="k", name=f"k_{b}_0")
                nc.sync.dma_start(kh0[:, 0:4], k_t[b, :, 0:4])
                qt_sb = qtp.tile([P, QC, DT, 512], BF, tag="q", name=f"q_{b}")
                nc.sync.dma_start(qt_sb[:, 0], q_t[b, :, 0])
                nc.sync.dma_start(qt_sb[:, 1], q_t[b, :, 1])
                _m_chunk(b, 0)
                _m_chunk(b, 2)
                nc.sync.dma_start(kh0[:, 4:KH], k_t[b, :, 4:KH])
                _m_chunk(b, 4)
                kh1 = ktp.tile([P, KH, DT, P], BF, tag="k", name=f"k_{b}_1")
                nc.sync.dma_start(kh1[:], k_t[b, :, KH:KT])
                _m_chunk(b, 6)
                _m_chunk(b, 8)
                v_sb = vp.tile([P, KT, V], BF, tag="v", name=f"v_{b}")
                nc.sync.dma_start(v_sb[:, 0:KH], v_t[b, :, 0:KH])
                _m_chunk(b, 10)
                nc.sync.dma_start(v_sb[:, KH:KT], v_t[b, :, KH:KT])
                _m_chunk(b, 12)
                _m_chunk(b, 14)
                nc.sync.dma_start(w_sb[:], w_t[:])
                nc.sync.dma_start(bias_sb[:], b_r[:])
                ks[b], qts[b], vs[b] = (kh0, kh1), qt_sb, v_sb

            def stage1():
                # batch 1: bulk loads first (they have until ph1(1)/ph2(1)),
                # then mask chunks, which self-pace through the pool slots.
                b = 1
                ms[b] = []
                kh0 = ktp.tile([P, KH, DT, P], BF, tag="k", name=f"k_{b}_0")
                nc.sync.dma_start(kh0[:], k_t[b, :, 0:KH])
                qt_sb = qtp.tile([P, QC, DT, 512], BF, tag="q", name=f"q_{b}")
                nc.sync.dma_start(qt_sb[:], q_t[b])
                kh1 = ktp.tile([P, KH, DT, P], BF, tag="k", name=f"k_{b}_1")
                nc.sync.dma_start(kh1[:], k_t[b, :, KH:KT])
                v_sb = vp.tile([P, KT, V], BF, tag="v", name=f"v_{b}")
                nc.sync.dma_start(v_sb[:], v_t[b])
                for kt0 in range(0, KT, MC):
                    _m_chunk(b, kt0)
                ks[b], qts[b], vs[b] = (kh0, kh1), qt_sb, v_sb

            state = {}

            def ph1(b):
                khs, qt_sb, mrows = ks[b], qts[b], ms[b]
                expt = exp_p.tile([P, KT, NQ], BF, tag="expt", name=f"expt_{b}")
                acc = accp.tile([P, NQ], F32, tag="acc", name=f"acc_{b}")
                levels = {}

                def tree_push(lv, t):
                    # pairwise DVE add-tree over exp tiles, bf16 until the
                    # final fp32 combine into acc
                    while lv in levels:
                        t2 = levels.pop(lv)
                        if lv == 3:
                            nc.vector.tensor_tensor(acc[:], t2, t, ADD)
                            return
                        pool = etp if lv == 0 else (l2p if lv == 1 else l3p)
                        nt = pool.tile([P, NQ], BF, tag="etmp" if lv == 0 else "p")
                        nc.vector.tensor_tensor(nt[:], t2, t, ADD)
                        t, lv = nt[:], lv + 1
                    levels[lv] = t

                for kt in range(KT):
                    kh = khs[kt // KH]
                    ps = psp.tile([P, QC, 512], F32, tag="scores")
                    for dt in range(DT):
                        for qc in range(QC):
                            nc.tensor.matmul(
                                ps[:, qc, :],
                                kh[:, kt % KH, dt, :],
                                qt_sb[:, qc, dt, :],
                                start=(dt == 0),
                                stop=(dt == DT - 1),
                            )
                    et = etp.tile([P, NQ], BF, tag="etmp")
                    for qc in range(QC):
                        nc.scalar.activation(
                            et[:, qc * 512:(qc + 1) * 512], ps[:, qc, :],
                            mybir.ActivationFunctionType.Exp, scale=float(SCALE),
                        )
                    nc.vector.tensor_tensor(expt[:, kt, :], et[:], mrows[kt], MULT)
                    tree_push(0, expt[:, kt, :])
                state[b] = (expt, acc)

            def den_recip(b):
                acc = state[b][1]
                den_ps = pop.tile([P, 2, O], F32, tag="out")
                for qt in range(QT):
                    nc.tensor.matmul(
                        den_ps[0:P, 0, qt:qt + 1],
                        acc[:, qt * P:(qt + 1) * P],
                        ones_sb[:],
                        start=True,
                        stop=True,
                    )
                recips = rcp.tile([P, QT], F32, tag="recips", name=f"recips_{b}")
                nc.vector.reciprocal(recips[:], den_ps[:, 0, 0:QT])
                state[b] = (state[b][0], recips)

            def ph2_qc(b, qc, ctxt):
                expt = state[b][0]
                v_sb = vs[b]
                for vt in range(VT):
                    ps_c = pcp.tile([P, 512], F32, tag="ctx")
                    for kt in range(KT):
                        nc.tensor.matmul(
                            ps_c[:],
                            v_sb[:, kt, vt * P:(vt + 1) * P],
                            expt[:, kt, qc * 512:(qc + 1) * 512],
                            start=(kt == 0),
                            stop=(kt == KT - 1),
                        )
                    nc.vector.tensor_copy(ctxt[:, vt, qc * 512:(qc + 1) * 512], ps_c[:])

            def ph3_half(b, qc, ctxt):
                recips = state[b][1]
                for qh in range(QT // QC // 2):  # 2 qt-pairs per q-chunk
                    ps_o = pop.tile([P, 2, O], F32, tag="out")
                    for s in range(2):
                        qt = qc * (QT // QC) + qh * 2 + s
                        for vt in range(VT):
                            nc.tensor.matmul(
                                ps_o[:, s, :],
                                ctxt[:, vt, qt * P:(qt + 1) * P],
                                w_sb[:, vt, :],
                                start=(vt == 0),
                                stop=(vt == VT - 1),
                            )
                        out_t = osp.tile([P, O], F32, tag="out_sb")
                        nc.vector.scalar_tensor_tensor(
                            out_t[:], ps_o[:, s, :], recips[:, qt:qt + 1], bias_sb[:],
                            MULT, ADD,
                        )
                        nc.gpsimd.dma_start(out[b, qt * P:(qt + 1) * P, :], out_t[:])

            def ph23(b):
                ctxt = ctp.tile([P, VT, NQ], BF, tag="ctxt", name=f"ctxt_{b}")
                ph2_qc(b, 0, ctxt)
                den_recip(b)
                ph3_half(b, 0, ctxt)
                ph2_qc(b, 1, ctxt)
                ph3_half(b, 1, ctxt)

            stage0()
            stage1()
            ph1(0)
            ph23(0)
            ph1(1)
            ph23(1)

    nc.finalize()
    return nc


def _prep(keys, queries, values, mask, W_resize, b_resize):
    bf = mybir.dt.np(BF)
    k_bf = np.asarray(keys, dtype=np.float32).astype(bf)
    q_bf = np.asarray(queries, dtype=np.float32).astype(bf)
    v_bf = np.asarray(values, dtype=np.float32).astype(bf)
    m_bf = np.asarray(mask).astype(bf)
    w_bf = np.asarray(W_resize, dtype=np.float32).astype(bf)

    # k_t[b, p, kt, dt, c] = K[b, kt*128+c, dt*128+p]
    k_t = np.ascontiguousarray(
        k_bf.reshape(B, KT, P, DT, P).transpose(0, 4, 1, 3, 2)
    )
    # q_t[b, p, qc, dt, j] = Q[b, qc*512+j, dt*128+p]
    q_t = np.ascontiguousarray(
        q_bf.reshape(B, QC, 512, DT, P).transpose(0, 4, 1, 3, 2)
    )
    # v_t[b, p, kt, v] = V[b, kt*128+p, v]
    v_t = np.ascontiguousarray(v_bf.reshape(B, KT, P, V).transpose(0, 2, 1, 3))
    # m_t[b, p, kt, q] = mask[b, q, kt*128+p]
    m_t = np.ascontiguousarray(m_bf.reshape(B, NQ, KT, P).transpose(0, 3, 2, 1))
    # w_t[p, vt, o] = W[o, vt*128+p]
    w_t = np.ascontiguousarray(w_bf.reshape(O, VT, P).transpose(2, 1, 0))
    b_rep = np.ascontiguousarray(
        np.broadcast_to(np.asarray(b_resize, dtype=np.float32).reshape(1, O), (P, O))
    )
    return k_t, q_t, v_t, m_t, w_t, b_rep


def kernel(keys, queries, values, mask, W_resize, b_resize):
    k_t, q_t, v_t, m_t, w_t, b_rep = _prep(
        keys, queries, values, mask, W_resize, b_resize
    )

    if "nc" not in _NC_CACHE:
        _NC_CACHE["nc"] = _build()
    nc = _NC_CACHE["nc"]

    in_maps = []
    for c in range(N_CORES):
        s = slice(c * B_LOC, (c + 1) * B_LOC)
        in_maps.append(
            {
                "k_t": k_t[s],
                "q_t": q_t[s],
                "v_t": v_t[s],
                "m_t": m_t[s],
                "w_t": w_t,
                "b_resize": b_rep,
            }
        )

    r = run_bass_kernel_spmd(nc, in_maps, list(range(N_CORES)))
    _LAST_RESULTS["r"] = r
    return np.concatenate([r.results[c]["out"] for c in range(N_CORES)], axis=0)


# revision 15
# speedup vs baseline: 1.0560x; 1.0099x over previous
"""Bass/Trainium2 kernel for masked attention + resize (nn_BaseAttender).

Full-input contract: kernel(**inputs) takes the complete unsharded tensors,
shards batch-wise across 8 NeuronCores (2 batches per core), runs one SPMD
Bass program, and gathers the full [16, 1024, 256] output.

Math (per batch):
    logits  = Q @ K^T / sqrt(512)              [1024, 2048]
    attn    = softmax(where(mask==0, -1e9, logits))
    context = attn @ V                          [1024, 512]
    out     = context @ W^T + b                 [1024, 256]

Implementation notes:
  - all heavy-lift layout work is done on the HOST: Q^T, K^T, mask^T are
    pre-transposed and pre-cast to bf16 so the device does ZERO PE
    transposes and loads exactly the tiles it consumes via big contiguous
    DMAs.  This kernel's PE time is ~pure GEMM.
  - scores are computed directly in [k, q] layout (stationary = K^T tile,
    moving = Q^T), so exp*mask is already in the layout phase 2 needs.
  - softmax without max-subtraction: logits are O(5) so exp() is safe, and
    where(mask==0,-inf)+softmax == exp(logits)*mask normalized by its sum.
  - denominator: a pairwise bf16 add-tree on the DVE accumulates
    sum_kt exp tiles (GPSIMD shares SBUF ports with the DVE and slows it
    4x, so the tree must stay on the DVE); a 1-moving-column fp32 matmul
    per q-tile (stationary = acc slice, moving = ones) reduces over
    partitions directly into [q, 1] layout.
  - phase 3 (resize) is interleaved into phase 2 per 512-query chunk so
    its small PSUM groups hide behind phase-2 streaming.
  - a warmup matmul group runs during the input-DMA wait to bring the PE
    out of its low p-state before the first real matmul.
  - the 1/denominator scaling commutes past the k- and v-contractions and
    is applied once at the very end on [q, 256] tiles.
"""

import sys

sys.path.insert(0, "/opt/trn_rl_repo")

import numpy as np

import concourse.bass as bass
import concourse.tile as tile
from concourse import bacc, mybir
from concourse.bass_utils import run_bass_kernel_spmd

# problem shape (hardcoded per contract)
B, NQ, NK, D, V, O = 16, 1024, 2048, 512, 512, 256
N_CORES = 8
B_LOC = B // N_CORES          # batches per core
SCALE = 1.0 / np.sqrt(np.float32(512.0))

P = 128
DT = D // P                   # 4 d-tiles (contraction of phase 1)
KT = NK // P                  # 16 k-tiles
QT = NQ // P                  # 8 q-tiles
QC = NQ // 512                # 2 q-chunks of 512 (phase-1 moving dim)
VT = V // P                   # 4 v-tiles
MC = 2                        # k-tiles per mask DMA chunk
NMC = KT // MC                # 8 mask chunks per batch
KH = KT // 2                  # k-tiles per K^T half-tile

F32 = mybir.dt.float32
BF = mybir.dt.bfloat16

_NC_CACHE = {}
_LAST_RESULTS = {}

ADD = mybir.AluOpType.add
MULT = mybir.AluOpType.mult


def _build():
    nc = bacc.Bacc(num_swdge_queues=2)
    k_t = nc.declare_dram_parameter("k_t", [B_LOC, P, KT, DT, P], BF, isOutput=False)
    q_t = nc.declare_dram_parameter("q_t", [B_LOC, P, QC, DT, 512], BF, isOutput=False)
    v_t = nc.declare_dram_parameter("v_t", [B_LOC, P, KT, V], BF, isOutput=False)
    m_t = nc.declare_dram_parameter("m_t", [B_LOC, P, KT, NQ], BF, isOutput=False)
    w_t = nc.declare_dram_parameter("w_t", [P, VT, O], BF, isOutput=False)
    b_r = nc.declare_dram_parameter("b_resize", [P, O], F32, isOutput=False)
    out = nc.declare_dram_parameter("out", [B_LOC, NQ, O], F32, isOutput=True)

    with tile.TileContext(nc) as tc:
        with (
            tc.tile_pool(name="const", bufs=1) as constp,
            tc.tile_pool(name="kt_sb", bufs=3) as ktp,
            tc.tile_pool(name="qt_sb", bufs=2) as qtp,
            tc.tile_pool(name="v_sb", bufs=2) as vp,
            tc.tile_pool(name="m_sb", bufs=3) as mp,
            tc.tile_pool(name="etmp", bufs=6) as etp,      # exp staging + L1 partials
            tc.tile_pool(name="l2p", bufs=2) as l2p,
            tc.tile_pool(name="l3p", bufs=2) as l3p,
            tc.tile_pool(name="expt", bufs=2) as exp_p,
            tc.tile_pool(name="acc", bufs=2) as accp,
            tc.tile_pool(name="ctxt", bufs=1) as ctp,
            tc.tile_pool(name="recips", bufs=2) as rcp,
            tc.tile_pool(name="outsb", bufs=4) as osp,
            tc.tile_pool(name="ps_s", bufs=2, space="PSUM") as psp,    # phase-1 scores
            tc.tile_pool(name="ps_c", bufs=2, space="PSUM") as pcp,    # phase-2 context
            tc.tile_pool(name="ps_o", bufs=2, space="PSUM") as pop,    # out + den + warmup
        ):
            w_sb = constp.tile([P, VT, O], BF)
            ones_bf = constp.tile([P, 1], BF)
            nc.vector.memset(ones_bf[:], 1.0)
            ones_sb = constp.tile([P, 1], F32)
            nc.vector.memset(ones_sb[:], 1.0)
            warmdata = constp.tile([P, 512], BF)
            nc.vector.memset(warmdata[:], 0.0)

            # PE warmup during the input-DMA wait: one long accumulation
            # group streaming a memset tile through the array (result
            # discarded, no DMA dependency).
            warm = pop.tile([P, 2, O], F32, tag="out")
            NWARM = 12
            for i in range(NWARM):
                nc.tensor.matmul(
                    warm[0:1, :, :], ones_bf[:], warmdata[:],
                    start=(i == 0), stop=(i == NWARM - 1),
                )

            bias_sb = constp.tile([P, O], F32)

            ks, qts, vs, ms = {}, {}, {}, {}

            def _m_chunk(b, kt0, n=MC):
                mrow = mp.tile([P, MC, NQ], BF, tag="m", name=f"m_{b}_{kt0}")
                nc.sync.dma_start(mrow[:, 0:n], m_t[b, :, kt0:kt0 + n])
                for i in range(n):
                    ms[b].append(mrow[:, i, :])

            # All input loads go through the single sync queue in strict
            # need order: same-queue transfers serialize in order, while
            # transfers from different queues packet-interleave on the DMA
            # rings (fair share) and would starve the critical first tiles.
            def stage0():
                b = 0
                ms[b] = []
                kh0 = ktp.tile([P, KH, DT, P], BF, tag="k", name=f"k_{b}_0")
                nc.sync.dma_start(kh0[:, 0:4], k_t[b, :, 0:4])
                qt_sb = qtp.tile([P, QC, DT, 512], BF, tag="q", name=f"q_{b}")
                nc.sync.dma_start(qt_sb[:, 0], q_t[b, :, 0])
                nc.sync.dma_start(qt_sb[:, 1], q_t[b, :, 1])
                _m_chunk(b, 0, 1)
                _m_chunk(b, 1, 1)
                _m_chunk(b, 2)
                nc.sync.dma_start(kh0[:, 4:KH], k_t[b, :, 4:KH])
                _m_chunk(b, 4)
                kh1 = ktp.tile([P, KH, DT, P], BF, tag="k", name=f"k_{b}_1")
                nc.sync.dma_start(kh1[:], k_t[b, :, KH:KT])
                _m_chunk(b, 6)
                _m_chunk(b, 8)
                v_sb = vp.tile([P, KT, V], BF, tag="v", name=f"v_{b}")
                nc.sync.dma_start(v_sb[:, 0:KH], v_t[b, :, 0:KH])
                _m_chunk(b, 10)
                nc.sync.dma_start(v_sb[:, KH:KT], v_t[b, :, KH:KT])
                _m_chunk(b, 12)
                _m_chunk(b, 14)
                nc.sync.dma_start(w_sb[:], w_t[:])
                nc.sync.dma_start(bias_sb[:], b_r[:])
                ks[b], qts[b], vs[b] = (kh0, kh1), qt_sb, v_sb

            def stage1():
                # batch 1: bulk loads first (they have until ph1(1)/ph2(1)),
                # then mask chunks, which self-pace through the pool slots.
                b = 1
                ms[b] = []
                kh0 = ktp.tile([P, KH, DT, P], BF, tag="k", name=f"k_{b}_0")
                nc.sync.dma_start(kh0[:], k_t[b, :, 0:KH])
                qt_sb = qtp.tile([P, QC, DT, 512], BF, tag="q", name=f"q_{b}")
                nc.sync.dma_start(qt_sb[:], q_t[b])
                kh1 = ktp.tile([P, KH, DT, P], BF, tag="k", name=f"k_{b}_1")
                nc.sync.dma_start(kh1[:], k_t[b, :, KH:KT])
                v_sb = vp.tile([P, KT, V], BF, tag="v", name=f"v_{b}")
                nc.sync.dma_start(v_sb[:], v_t[b])
                for kt0 in range(0, KT, MC):
                    _m_chunk(b, kt0)
                ks[b], qts[b], vs[b] = (kh0, kh1), qt_sb, v_sb

            state = {}

            def ph1(b):
                khs, qt_sb, mrows = ks[b], qts[b], ms[b]
                expt = exp_p.tile([P, KT, NQ], BF, tag="expt", name=f"expt_{b}")
                acc = accp.tile([P, NQ], F32, tag="acc", name=f"acc_{b}")
                levels = {}

                def tree_push(lv, t):
                    # pairwise DVE add-tree over exp tiles, bf16 until the
                    # final fp32 combine into acc
                    while lv in levels:
                        t2 = levels.pop(lv)
                        if lv == 3:
                            nc.vector.tensor_tensor(acc[:], t2, t, ADD)
                            return
                        pool = etp if lv == 0 else (l2p if lv == 1 else l3p)
                        nt = pool.tile([P, NQ], BF, tag="etmp" if lv == 0 else "p")
                        nc.vector.tensor_tensor(nt[:], t2, t, ADD)
                        t, lv = nt[:], lv + 1
                    levels[lv] = t

                for kt in range(KT):
                    kh = khs[kt // KH]
                    ps = psp.tile([P, QC, 512], F32, tag="scores")
                    for dt in range(DT):
                        for qc in range(QC):
                            nc.tensor.matmul(
                                ps[:, qc, :],
                                kh[:, kt % KH, dt, :],
                                qt_sb[:, qc, dt, :],
                                start=(dt == 0),
                                stop=(dt == DT - 1),
                            )
                    et = etp.tile([P, NQ], BF, tag="etmp")
                    for qc in range(QC):
                        nc.scalar.activation(
                            et[:, qc * 512:(qc + 1) * 512], ps[:, qc, :],
                            mybir.ActivationFunctionType.Exp, scale=float(SCALE),
                        )
                    nc.vector.tensor_tensor(expt[:, kt, :], et[:], mrows[kt], MULT)
                    tree_push(0, expt[:, kt, :])
                state[b] = (expt, acc)

            def den_recip(b):
                acc = state[b][1]
                den_ps = pop.tile([P, 2, O], F32, tag="out")
                for qt in range(QT):
                    nc.tensor.matmul(
                        den_ps[0:P, 0, qt:qt + 1],
                        acc[:, qt * P:(qt + 1) * P],
                        ones_sb[:],
                        start=True,
                        stop=True,
                    )
                recips = rcp.tile([P, QT], F32, tag="recips", name=f"recips_{b}")
                nc.vector.reciprocal(recips[:], den_ps[:, 0, 0:QT])
                state[b] = (state[b][0], recips)

            def ph2_qc(b, qc, ctxt):
                expt = state[b][0]
                v_sb = vs[b]
                for vt in range(VT):
                    ps_c = pcp.tile([P, 512], F32, tag="ctx")
                    for kt in range(KT):
                        nc.tensor.matmul(
                            ps_c[:],
                            v_sb[:, kt, vt * P:(vt + 1) * P],
                            expt[:, kt, qc * 512:(qc + 1) * 512],
                            start=(kt == 0),
                            stop=(kt == KT - 1),
                        )
                    nc.vector.tensor_copy(ctxt[:, vt, qc * 512:(qc + 1) * 512], ps_c[:])

            def ph3_half(b, qc, ctxt):
                recips = state[b][1]
                for qh in range(QT // QC // 2):  # 2 qt-pairs per q-chunk
                    ps_o = pop.tile([P, 2, O], F32, tag="out")
                    for s in range(2):
                        qt = qc * (QT // QC) + qh * 2 + s
                        for vt in range(VT):
                            nc.tensor.matmul(
                                ps_o[:, s, :],
                                ctxt[:, vt, qt * P:(qt + 1) * P],
                                w_sb[:, vt, :],
                                start=(vt == 0),
                                stop=(vt == VT - 1),
                            )
                        out_t = osp.tile([P, O], F32, tag="out_sb")
                        nc.vector.scalar_tensor_tensor(
                            out_t[:], ps_o[:, s, :], recips[:, qt:qt + 1], bias_sb[:],
                            MULT, ADD,
                        )
                        nc.scalar.dma_start(out[b, qt * P:(qt + 1) * P, :], out_t[:])

            def ph23(b):
                ctxt = ctp.tile([P, VT, NQ], BF, tag="ctxt", name=f"ctxt_{b}")
                ph2_qc(b, 0, ctxt)
                den_recip(b)
                ph3_half(b, 0, ctxt)
                ph2_qc(b, 1, ctxt)
                ph3_half(b, 1, ctxt)

            stage0()
            stage1()
            ph1(0)
            ph23(0)
            ph1(1)
            ph23(1)

    nc.finalize()
    return nc


def _prep(keys, queries, values, mask, W_resize, b_resize):
    bf = mybir.dt.np(BF)
    k_bf = np.asarray(keys, dtype=np.float32).astype(bf)
    q_bf = np.asarray(queries, dtype=np.float32).astype(bf)
    v_bf = np.asarray(values, dtype=np.float32).astype(bf)
    m_bf = np.asarray(mask).astype(bf)
    w_bf = np.asarray(W_resize, dtype=np.float32).astype(bf)

    # k_t[b, p, kt, dt, c] = K[b, kt*128+c, dt*128+p]
    k_t = np.ascontiguousarray(
        k_bf.reshape(B, KT, P, DT, P).transpose(0, 4, 1, 3, 2)
    )
    # q_t[b, p, qc, dt, j] = Q[b, qc*512+j, dt*128+p]
    q_t = np.ascontiguousarray(
        q_bf.reshape(B, QC, 512, DT, P).transpose(0, 4, 1, 3, 2)
    )
    # v_t[b, p, kt, v] = V[b, kt*128+p, v]
    v_t = np.ascontiguousarray(v_bf.reshape(B, KT, P, V).transpose(0, 2, 1, 3))
    # m_t[b, p, kt, q] = mask[b, q, kt*128+p]
    m_t = np.ascontiguousarray(m_bf.reshape(B, NQ, KT, P).transpose(0, 3, 2, 1))
    # w_t[p, vt, o] = W[o, vt*128+p]
    w_t = np.ascontiguousarray(w_bf.reshape(O, VT, P).transpose(2, 1, 0))
    b_rep = np.ascontiguousarray(
        np.broadcast_to(np.asarray(b_resize, dtype=np.float32).reshape(1, O), (P, O))
    )
    return k_t, q_t, v_t, m_t, w_t, b_rep


def kernel(keys, queries, values, mask, W_resize, b_resize):
    k_t, q_t, v_t, m_t, w_t, b_rep = _prep(
        keys, queries, values, mask, W_resize, b_resize
    )

    if "nc" not in _NC_CACHE:
        _NC_CACHE["nc"] = _build()
    nc = _NC_CACHE["nc"]

    in_maps = []
    for c in range(N_CORES):
        s = slice(c * B_LOC, (c + 1) * B_LOC)
        in_maps.append(
            {
                "k_t": k_t[s],
                "q_t": q_t[s],
                "v_t": v_t[s],
                "m_t": m_t[s],
                "w_t": w_t,
                "b_resize": b_rep,
            }
        )

    r = run_bass_kernel_spmd(nc, in_maps, list(range(N_CORES)))
    _LAST_RESULTS["r"] = r
    return np.concatenate([r.results[c]["out"] for c in range(N_CORES)], axis=0)
